# revision 23
# baseline (speedup 1.0000x reference)
"""Single-head causal attention on 8 Trainium2 NeuronCores.

B=4, T=4096, E=1024, H=128, fp32 in/out.

Sharding: batch-parallel x query-parallel. Two programs (one per query half):
  program A cores (devices 0-3): batch d, queries [0:1024) u [3072:4096)
  program B cores (devices 4-7): batch d-4, queries [1024:3072)
Per-core matmul unit counts (128x128x512 streams) balance at 296/296.

All DMAs are host-side pre-packed to be contiguous per partition:
  xp   [kv_ccs*NE*128, 512] fp16: block (cc,e) rows = x[b, cc*512+t, e*128+p]
  w*   [128, NE*H]  fp16: w[p, e*H+h] = W[e*128+p, h]
  out  [NT*128, 512] fp32: tile j rows p, cols c*128+h = O[t0_j + c*128 + p, h]

On-chip (per core); matmul operands fp16, accumulation fp32:
  0. PE warmup matmuls (identity) fill the initial DMA-wait window so the
     HAM clock gate reaches 8/8 before the real stream begins.
  1. QKV projections per 512-column chunk, 8 E-steps accumulated in PSUM
     ([128,1024] pair tiles hold K|V halves). V natural via PE transposes
     batched 4-per-bank then one DVE copy.
  2. Per 512-query tile, key tiles of 128 processed in PAIRS (diagonal
     pair first): two S^T matmuls into one [128,1024] PSUM pair, ONE exp
     (ScalarE) over 1024 cols -> fp16 SBUF, causal mask on diagonal tiles
     via gpsimd affine_select on halves, two PV matmuls accumulate into
     OT PSUM; G0 += half0 / G1 += half1 on DVE (fp16 2x mode).
  3. denom = colsum(G0)+colsum(G1) via ones-matmuls into PSUM, DVE
     reciprocal, PE-transpose OT chunks, row-scale by 1/denom, one
     contiguous DMA per query tile -> out [128, 512] fp32.
"""

import numpy as np

import concourse.bass as bass
import concourse.bacc as bacc
import concourse.mybir as mybir
import concourse.tile as tile
from concourse.masks import make_identity

B, T, E, H = 4, 4096, 1024, 128
TQ = 512          # query tile width
NE = E // 128     # 8 e-chunks
QROWS = 2048      # queries per core
NT = QROWS // TQ  # 4 query tiles per core
SCALE = float(H) ** -0.5
F32 = mybir.dt.float32
F16 = mybir.dt.float16

T0S_A = [0, 512, 2560, 3584]
T0S_B = [1024, 1536, 2048, 3072]
N_WARM = 36


def _build(t0s):
    nc = bacc.Bacc("TRN2", target_bir_lowering=False, debug=False, num_devices=4)
    kv_ccs = max((t0 + TQ) // TQ for t0 in t0s)       # 8 for A, 7 for B
    kv_cols = kv_ccs * TQ
    xp = nc.declare_dram_parameter("xp", [kv_ccs * 128, NE * TQ], F16, isOutput=False)
    Wq = nc.declare_dram_parameter("Wq", [128, NE * H], F16, isOutput=False)
    Wk = nc.declare_dram_parameter("Wk", [128, NE * H], F16, isOutput=False)
    Wv = nc.declare_dram_parameter("Wv", [128, NE * H], F16, isOutput=False)
    out = nc.declare_dram_parameter("out", [NT * 128, TQ], F32, isOutput=True)

    qcc = {t0 // TQ: j for j, t0 in enumerate(t0s)}  # T col-chunk -> q tile slot

    with tile.TileContext(nc) as tc:
        with (
            tc.tile_pool(name="const", bufs=1) as const_pool,
            tc.tile_pool(name="wts", bufs=1) as wt_pool,
            tc.tile_pool(name="big", bufs=1) as big_pool,
            tc.tile_pool(name="ev", bufs=6) as e_pool,
            tc.tile_pool(name="g", bufs=2) as g_pool,
            tc.tile_pool(name="ot", bufs=2) as ot_pool,
            tc.tile_pool(name="small", bufs=4) as small_pool,
            tc.tile_pool(name="onat", bufs=2) as onat_pool,
            tc.tile_pool(name="mmp", bufs=3, space="PSUM") as mm_psum,
            tc.tile_pool(name="pv", bufs=2, space="PSUM") as pv_psum,
            tc.tile_pool(name="tpv", bufs=1, space="PSUM") as tpv_psum,
            tc.tile_pool(name="misc", bufs=2, space="PSUM") as misc_psum,
        ):
            ident16 = const_pool.tile([128, 128], F16, tag="id16")
            make_identity(nc, ident16[:])
            ident32 = const_pool.tile([128, 128], F32, tag="id32")
            make_identity(nc, ident32[:])
            ones = const_pool.tile([128, 1], F16, tag="ones")
            nc.gpsimd.memset(ones[:], 1.0)

            # PE warmup: keep the array busy through the HAM window while
            # the first input DMAs land.
            warm_ps = tpv_psum.tile([128, TQ], F32, tag="tpv", name="warm")
            for i in range(N_WARM):
                nc.tensor.matmul(
                    warm_ps[:, 0:128], ident16[:], ident16[:],
                    start=(i == 0), stop=(i == N_WARM - 1),
                )

            # weights (host pre-packed): single contiguous DMA each
            wq_sb = wt_pool.tile([128, NE * H], F16, tag="wq")
            wk_sb = wt_pool.tile([128, NE * H], F16, tag="wk")
            wv_sb = wt_pool.tile([128, NE * H], F16, tag="wv")
            nc.sync.dma_start(out=wk_sb[:], in_=Wk[:])

            # x tiers (host pre-packed, contiguous rows): one DMA per tier,
            # all on the sync DGE in consumption order -- per-queue FIFO
            # means tier k's descriptors complete before tier k+1's, so the
            # first tiers land at full HBM bandwidth. Tier 0 is split so the
            # first projection matmuls can start even earlier.
            xts = []  # per cc: list of (e_lo, n_e, tile)
            for cc in range(kv_ccs):
                parts = ((0, 2), (2, 2), (4, 2), (6, 2)) if cc == 0 else ((0, NE),)
                tiles = []
                for e_lo, n_e in parts:
                    xt = big_pool.tile([128, n_e * TQ], F16,
                                       tag=f"xt{cc}_{e_lo}")
                    r0 = cc * 128
                    nc.sync.dma_start(
                        out=xt[:],
                        in_=xp[r0:r0 + 128,
                               e_lo * TQ:(e_lo + n_e) * TQ],
                    )
                    tiles.append((e_lo, n_e, xt))
                xts.append(tiles)
                if cc == 0:
                    # wv/wq land after tier0 (V/Q projections run later)
                    nc.sync.dma_start(out=wv_sb[:], in_=Wv[:])
                    nc.scalar.dma_start(out=wq_sb[:], in_=Wq[:])

            def xslice(cc, e):
                for e_lo, n_e, xt in xts[cc]:
                    if e_lo <= e < e_lo + n_e:
                        return xt[:, (e - e_lo) * TQ:(e - e_lo + 1) * TQ]
                raise AssertionError

            KT = big_pool.tile([128, kv_cols], F16, tag="kt")   # K^T
            VT = big_pool.tile([128, kv_cols], F16, tag="vt")   # V^T
            V = big_pool.tile([128, kv_cols], F16, tag="v")     # V natural
            QT = big_pool.tile([128, QROWS], F16, tag="qt")     # Q^T

            def project(cc):
                c0 = cc * TQ
                for w_sb, dstbuf, d0, cp in (
                    (wk_sb, KT, c0, "scalar" if cc % 2 == 0 else "vector"),
                    (wv_sb, VT, c0, "vector" if cc % 2 == 0 else "scalar"),
                ) + (((wq_sb, QT, qcc[cc] * TQ, "vector"),) if cc in qcc else ()):
                    ps = mm_psum.tile([128, TQ], F32, tag="mm", name="ps")
                    for e in range(NE):
                        nc.tensor.matmul(
                            ps[:], w_sb[:, e * H:(e + 1) * H], xslice(cc, e),
                            start=(e == 0), stop=(e == NE - 1),
                        )
                    if cp == "scalar":
                        nc.scalar.copy(dstbuf[:, d0:d0 + TQ], ps[:])
                    else:
                        nc.vector.tensor_copy(dstbuf[:, d0:d0 + TQ], ps[:])
                # V natural: 4 PE transposes into one bank, one DVE copy out
                tp16 = tpv_psum.tile([128, TQ], F16, tag="tpv", name="tp16")
                for i in range(4):
                    kt = cc * 4 + i
                    nc.tensor.transpose(
                        tp16[:, i * 128:(i + 1) * 128],
                        VT[:, kt * 128:(kt + 1) * 128], ident16[:],
                    )
                nc.vector.tensor_copy(V[:, c0:c0 + TQ], tp16[:])

            def attend(j, t0):
                nkt = (t0 + TQ) // 128
                diag0 = t0 // 128
                ot_ps = pv_psum.tile([128, TQ], F32, name="ot_ps")
                G0 = g_pool.tile([128, TQ], F16, tag="g0", name="G0")
                G1 = g_pool.tile([128, TQ], F16, tag="g1", name="G1")
                kts = list(range(nkt - 1, -1, -1))  # diagonal first
                for i, kt in enumerate(kts):
                    st = mm_psum.tile([128, TQ], F32, tag="mm", name="st")
                    nc.tensor.matmul(
                        st[:], KT[:, kt * 128:(kt + 1) * 128],
                        QT[:, j * TQ:(j + 1) * TQ],
                        start=True, stop=True,
                    )
                    e_t = e_pool.tile([128, TQ], F16, name="e_t")
                    nc.scalar.activation(
                        e_t[:], st[:], mybir.ActivationFunctionType.Exp,
                        scale=SCALE,
                    )
                    if kt >= diag0:
                        # keep E[p,c] iff (t0+c) - (128*kt+p) >= 0
                        nc.gpsimd.affine_select(
                            out=e_t[:], in_=e_t[:],
                            compare_op=mybir.AluOpType.is_ge,
                            fill=0.0, base=t0 - 128 * kt,
                            pattern=[[1, TQ]], channel_multiplier=-1,
                        )
                    nc.tensor.matmul(
                        ot_ps[:], V[:, kt * 128:(kt + 1) * 128], e_t[:],
                        start=(i == 0), stop=(i == nkt - 1),
                    )
                    if i < 2:
                        nc.vector.tensor_copy((G0 if i == 0 else G1)[:], e_t[:])
                    elif i % 2 == 0:
                        nc.vector.tensor_add(G0[:], G0[:], e_t[:])
                    else:
                        nc.vector.tensor_add(G1[:], G1[:], e_t[:])

                ot_sb = ot_pool.tile([128, TQ], F32, name="ot_sb")
                nc.vector.tensor_copy(ot_sb[:], ot_ps[:])
                onat = onat_pool.tile([128, 4 * 128], F32, name="onat")
                dps = misc_psum.tile([128, 128], F32, tag="misc", name="dps")
                for c in range(TQ // 128):
                    nc.tensor.matmul(
                        dps[:, c:c + 1], G0[:, c * 128:(c + 1) * 128], ones[:],
                        start=True, stop=False,
                    )
                    nc.tensor.matmul(
                        dps[:, c:c + 1], G1[:, c * 128:(c + 1) * 128], ones[:],
                        start=False, stop=True,
                    )
                rc = small_pool.tile([128, 4], F32, name="rc")
                nc.vector.reciprocal(rc[:], dps[:, 0:4])
                for c in range(TQ // 128):
                    tp = misc_psum.tile([128, 128], F32, tag="misc", name="tp")
                    nc.tensor.transpose(
                        tp[:], ot_sb[:, c * 128:(c + 1) * 128], ident32[:]
                    )
                    nc.vector.tensor_scalar_mul(
                        onat[:, c * 128:(c + 1) * 128], tp[:], rc[:, c:c + 1]
                    )
                # one contiguous DMA per query tile (host un-permutes)
                nc.sync.dma_start(
                    out=out[j * 128:(j + 1) * 128, :], in_=onat[:],
                )

            # interleave: project each chunk, then run any attention tile
            # whose keys/queries are now fully projected
            done = set()
            for cc in range(kv_ccs):
                project(cc)
                for j, t0 in enumerate(t0s):
                    if j in done:
                        continue
                    # one chunk of slack after strictly-ready to avoid PE
                    # head-of-line stalls on exp/mask deps
                    if (t0 + TQ) // TQ <= cc and (t0 // TQ) < cc or cc == kv_ccs - 1:
                        done.add(j)
                        attend(j, t0)
            assert done == set(range(len(t0s)))

    nc.finalize()
    return nc


# ---------------- host-side run ----------------

_CACHE = {}


def _runner(nc, devices):
    """run_bass_via_pjrt with an explicit device list (subset launch)."""
    import jax
    from jax.sharding import Mesh, PartitionSpec
    from jax.experimental.shard_map import shard_map
    from concourse.bass2jax import _bass_exec_p, install_neuronx_cc_hook

    install_neuronx_cc_hook()
    n_cores = len(devices)
    part_name = nc.partition_id_tensor.name if nc.partition_id_tensor else None
    in_names, out_names, out_avals, zero_outs = [], [], [], []
    for alloc in nc.m.functions[0].allocations:
        if not isinstance(alloc, mybir.MemoryLocationSet):
            continue
        name = alloc.memorylocations[0].name
        if alloc.kind == "ExternalInput":
            if name != part_name:
                in_names.append(name)
        elif alloc.kind == "ExternalOutput":
            shape = tuple(alloc.tensor_shape)
            dtype = mybir.dt.np(alloc.dtype)
            out_names.append(name)
            out_avals.append(jax.core.ShapedArray(shape, dtype))
            zero_outs.append(np.zeros(shape, dtype))
    n_params = len(in_names)
    n_outs = len(out_avals)
    in_names = in_names + out_names
    if part_name is not None:
        in_names = in_names + [part_name]
    donate = tuple(range(n_params, n_params + n_outs))

    def _body(*args):
        from concourse.bass2jax import partition_id_tensor
        operands = list(args)
        if part_name is not None:
            operands.append(partition_id_tensor())
        outs = _bass_exec_p.bind(
            *operands,
            out_avals=tuple(out_avals),
            in_names=tuple(in_names),
            out_names=tuple(out_names),
            lowering_input_output_aliases=(),
            sim_require_finite=True,
            sim_require_nnan=True,
            nc=nc,
        )
        return tuple(outs)

    mesh = Mesh(np.asarray(devices), ("core",))
    sharded = jax.jit(
        shard_map(
            _body, mesh=mesh,
            in_specs=(PartitionSpec("core"),) * (n_params + n_outs),
            out_specs=(PartitionSpec("core"),) * n_outs,
            check_rep=False,
        ),
        donate_argnums=donate, keep_unused=True,
    )

    def run(in_maps):
        per_core = [[np.asarray(m[n]) for n in in_names[:n_params]] for m in in_maps]
        concat_in = [
            np.concatenate([per_core[c][i] for c in range(n_cores)], axis=0)
            for i in range(n_params)
        ]
        concat_zeros = [
            np.zeros((n_cores * z.shape[0], *z.shape[1:]), z.dtype) for z in zero_outs
        ]
        return sharded(*concat_in, *concat_zeros)

    def finish(out_arrs):
        return [
            {
                n: np.asarray(out_arrs[i]).reshape(n_cores, *out_avals[i].shape)[c]
                for i, n in enumerate(out_names)
            }
            for c in range(n_cores)
        ]

    return run, finish


def _pack_x(xb, kv_ccs):
    """x[b] [T,E] fp32 -> [kv_ccs*128, NE*512] fp16, tier rows."""
    xv = xb[:kv_ccs * TQ].reshape(kv_ccs, TQ, NE, 128)
    return np.ascontiguousarray(
        xv.transpose(0, 3, 2, 1).reshape(kv_ccs * 128, NE * TQ)
    ).astype(np.float16)


def _pack_w(w):
    """W [E,H] -> [128, NE*H] fp16 with w[p, e*H+h] = W[e*128+p, h]."""
    return np.ascontiguousarray(
        w.reshape(NE, 128, H).transpose(1, 0, 2).reshape(128, NE * H)
    ).astype(np.float16)


def _get_runners():
    if "runners" not in _CACHE:
        import jax
        devs = jax.devices()
        ncA = _build(T0S_A)
        ncB = _build(T0S_B)
        _CACHE["ncs"] = (ncA, ncB)
        runA = _runner(ncA, devs[0:4])
        runB = _runner(ncB, devs[4:8])
        # Warm each executable once, sequentially and blocking, before
        # any concurrent use (cold concurrent dispatch has raced before).
        zs = []
        for ccs in (8, 7):
            zs.append([
                {
                    "xp": np.zeros((ccs * 128, NE * TQ), np.float16),
                    "Wq": np.zeros((128, NE * H), np.float16),
                    "Wk": np.zeros((128, NE * H), np.float16),
                    "Wv": np.zeros((128, NE * H), np.float16),
                }
                for _ in range(B)
            ])
        for (run, fin), z in zip((runA, runB), zs):
            fin(run(z))
        _CACHE["runners"] = (runA, runB)
    return _CACHE["runners"]


def kernel(x, Wq, Wk, Wv):
    x = np.asarray(x, dtype=np.float32)
    (runA, finA), (runB, finB) = _get_runners()

    wq16, wk16, wv16 = (_pack_w(np.asarray(w)) for w in (Wq, Wk, Wv))
    mapsA = [
        {"xp": _pack_x(x[b], 8), "Wq": wq16, "Wk": wk16, "Wv": wv16}
        for b in range(B)
    ]
    mapsB = [
        {"xp": _pack_x(x[b], 7), "Wq": wq16, "Wk": wk16, "Wv": wv16}
        for b in range(B)
    ]
    # dispatch both meshes before blocking on either
    outA = runA(mapsA)
    outB = runB(mapsB)
    resA = finA(outA)
    resB = finB(outB)

    full = np.empty((B, T, H), np.float32)
    for b in range(B):
        for res, t0list in ((resA[b], T0S_A), (resB[b], T0S_B)):
            o = res["out"].reshape(NT, 128, 4, H).transpose(0, 2, 1, 3)
            for j, t0 in enumerate(t0list):
                full[b, t0:t0 + TQ] = o[j].reshape(TQ, H)
    return full


# revision 25
# speedup vs baseline: 1.1234x; 1.1234x over previous
"""Single-head causal attention on 8 Trainium2 NeuronCores.

B=4, T=4096, E=1024, H=128, fp32 in/out.

Sharding: batch-parallel x query-parallel. Two programs (one per query half):
  program A cores (devices 0-3): batch d, queries [0:1024) u [3072:4096)
  program B cores (devices 4-7): batch d-4, queries [1024:3072)
Per-core matmul unit counts (128x128x512 streams) balance at 296/296.

All DMAs are host-side pre-packed to be contiguous per partition:
  xp   [kv_ccs*NE*128, 512] fp16: block (cc,e) rows = x[b, cc*512+t, e*128+p]
  w*   [128, NE*H]  fp16: w[p, e*H+h] = W[e*128+p, h]
  out  [NT*128, 512] fp32: tile j rows p, cols c*128+h = O[t0_j + c*128 + p, h]

On-chip (per core); matmul operands fp16, accumulation fp32:
  0. PE warmup matmuls (identity) fill the initial DMA-wait window so the
     HAM clock gate reaches 8/8 before the real stream begins.
  1. QKV projections per 512-column chunk, 8 E-steps accumulated in PSUM
     ([128,1024] pair tiles hold K|V halves). V natural via PE transposes
     batched 4-per-bank then one DVE copy.
  2. Per 512-query tile, key tiles of 128 processed in PAIRS (diagonal
     pair first): two S^T matmuls into one [128,1024] PSUM pair, ONE exp
     (ScalarE) over 1024 cols -> fp16 SBUF, causal mask on diagonal tiles
     via gpsimd affine_select on halves, two PV matmuls accumulate into
     OT PSUM; G0 += half0 / G1 += half1 on DVE (fp16 2x mode).
  3. denom = colsum(G0)+colsum(G1) via ones-matmuls into PSUM, DVE
     reciprocal, PE-transpose OT chunks, row-scale by 1/denom, one
     contiguous DMA per query tile -> out [128, 512] fp32.
"""

import numpy as np

import concourse.bass as bass
import concourse.bacc as bacc
import concourse.mybir as mybir
import concourse.tile as tile
from concourse.masks import make_identity

B, T, E, H = 4, 4096, 1024, 128
TQ = 512          # query tile width
NE = E // 128     # 8 e-chunks
QROWS = 2048      # queries per core
NT = QROWS // TQ  # 4 query tiles per core
SCALE = float(H) ** -0.5
F32 = mybir.dt.float32
F16 = mybir.dt.float16

T0S_A = [0, 512, 2560, 3584]
T0S_B = [1024, 1536, 2048, 3072]
N_WARM = 36


def _build(t0s):
    nc = bacc.Bacc("TRN2", target_bir_lowering=False, debug=False, num_devices=4)
    kv_ccs = max((t0 + TQ) // TQ for t0 in t0s)       # 8 for A, 7 for B
    kv_cols = kv_ccs * TQ
    xp = nc.declare_dram_parameter("xp", [kv_ccs * 128, NE * TQ], F16, isOutput=False)
    Wq = nc.declare_dram_parameter("Wq", [128, NE * H], F16, isOutput=False)
    Wk = nc.declare_dram_parameter("Wk", [128, NE * H], F16, isOutput=False)
    Wv = nc.declare_dram_parameter("Wv", [128, NE * H], F16, isOutput=False)
    out = nc.declare_dram_parameter("out", [NT * 128, TQ], F32, isOutput=True)

    qcc = {t0 // TQ: j for j, t0 in enumerate(t0s)}  # T col-chunk -> q tile slot

    with tile.TileContext(nc) as tc:
        with (
            tc.tile_pool(name="const", bufs=1) as const_pool,
            tc.tile_pool(name="wts", bufs=1) as wt_pool,
            tc.tile_pool(name="big", bufs=1) as big_pool,
            tc.tile_pool(name="ev", bufs=6) as e_pool,
            tc.tile_pool(name="g", bufs=2) as g_pool,
            tc.tile_pool(name="ot", bufs=2) as ot_pool,
            tc.tile_pool(name="small", bufs=4) as small_pool,
            tc.tile_pool(name="onat", bufs=2) as onat_pool,
            tc.tile_pool(name="mmp", bufs=4, space="PSUM") as mm_psum,
            tc.tile_pool(name="pv", bufs=2, space="PSUM") as pv_psum,
            tc.tile_pool(name="tpv", bufs=1, space="PSUM") as tpv_psum,
            tc.tile_pool(name="misc", bufs=1, space="PSUM") as misc_psum,
        ):
            ident16 = const_pool.tile([128, 128], F16, tag="id16")
            make_identity(nc, ident16[:])
            ident32 = const_pool.tile([128, 128], F32, tag="id32")
            make_identity(nc, ident32[:])
            ones = const_pool.tile([128, 1], F16, tag="ones")
            nc.gpsimd.memset(ones[:], 1.0)

            # PE warmup: keep the array busy through the HAM window while
            # the first input DMAs land.
            warm_ps = tpv_psum.tile([128, TQ], F32, tag="tpv", name="warm")
            for i in range(N_WARM):
                nc.tensor.matmul(
                    warm_ps[:, 0:128], ident16[:], ident16[:],
                    start=(i == 0), stop=(i == N_WARM - 1),
                )

            # weights (host pre-packed): single contiguous DMA each
            wq_sb = wt_pool.tile([128, NE * H], F16, tag="wq")
            wk_sb = wt_pool.tile([128, NE * H], F16, tag="wk")
            wv_sb = wt_pool.tile([128, NE * H], F16, tag="wv")
            nc.sync.dma_start(out=wk_sb[:], in_=Wk[:])

            # x tiers (host pre-packed, contiguous rows): one DMA per tier,
            # all on the sync DGE in consumption order -- per-queue FIFO
            # means tier k's descriptors complete before tier k+1's, so the
            # first tiers land at full HBM bandwidth. Tier 0 is split so the
            # first projection matmuls can start even earlier.
            xts = []  # per cc: list of (e_lo, n_e, tile)
            for cc in range(kv_ccs):
                parts = ((0, 2), (2, 2), (4, 4)) if cc == 0 else ((0, NE),)
                tiles = []
                for e_lo, n_e in parts:
                    xt = big_pool.tile([128, n_e * TQ], F16,
                                       tag=f"xt{cc}_{e_lo}")
                    r0 = cc * 128
                    nc.sync.dma_start(
                        out=xt[:],
                        in_=xp[r0:r0 + 128,
                               e_lo * TQ:(e_lo + n_e) * TQ],
                    )
                    tiles.append((e_lo, n_e, xt))
                xts.append(tiles)
                if cc == 0:
                    # wv/wq land after tier0 (V/Q projections run later)
                    nc.sync.dma_start(out=wv_sb[:], in_=Wv[:])
                    nc.scalar.dma_start(out=wq_sb[:], in_=Wq[:])

            def xslice(cc, e):
                for e_lo, n_e, xt in xts[cc]:
                    if e_lo <= e < e_lo + n_e:
                        return xt[:, (e - e_lo) * TQ:(e - e_lo + 1) * TQ]
                raise AssertionError

            KT = big_pool.tile([128, kv_cols], F16, tag="kt")   # K^T
            VT = big_pool.tile([128, kv_cols], F16, tag="vt")   # V^T
            V = big_pool.tile([128, kv_cols], F16, tag="v")     # V natural
            QT = big_pool.tile([128, QROWS], F16, tag="qt")     # Q^T

            def project(cc):
                c0 = cc * TQ
                for w_sb, dstbuf, d0, cp in (
                    (wk_sb, KT, c0, "scalar" if cc % 2 == 0 else "vector"),
                    (wv_sb, VT, c0, "vector" if cc % 2 == 0 else "scalar"),
                ) + (((wq_sb, QT, qcc[cc] * TQ, "vector"),) if cc in qcc else ()):
                    ps = mm_psum.tile([128, TQ], F32, tag="mm", name="ps")
                    for e in range(NE):
                        nc.tensor.matmul(
                            ps[:], w_sb[:, e * H:(e + 1) * H], xslice(cc, e),
                            start=(e == 0), stop=(e == NE - 1),
                        )
                    if cp == "scalar":
                        nc.scalar.copy(dstbuf[:, d0:d0 + TQ], ps[:])
                    else:
                        nc.vector.tensor_copy(dstbuf[:, d0:d0 + TQ], ps[:])
                # V natural: 4 PE transposes into one bank, one DVE copy out
                tp16 = tpv_psum.tile([128, TQ], F16, tag="tpv", name="tp16")
                for i in range(4):
                    kt = cc * 4 + i
                    nc.tensor.transpose(
                        tp16[:, i * 128:(i + 1) * 128],
                        VT[:, kt * 128:(kt + 1) * 128], ident16[:],
                    )
                nc.vector.tensor_copy(V[:, c0:c0 + TQ], tp16[:])

            def attend(j, t0):
                nkt = (t0 + TQ) // 128
                diag0 = t0 // 128
                ot_ps = pv_psum.tile([128, TQ], F32, name="ot_ps")
                G0 = g_pool.tile([128, TQ], F16, tag="g0", name="G0")
                G1 = g_pool.tile([128, TQ], F16, tag="g1", name="G1")
                kts = list(range(nkt - 1, -1, -1))  # diagonal first
                for i, kt in enumerate(kts):
                    st = mm_psum.tile([128, TQ], F32, tag="mm", name="st")
                    nc.tensor.matmul(
                        st[:], KT[:, kt * 128:(kt + 1) * 128],
                        QT[:, j * TQ:(j + 1) * TQ],
                        start=True, stop=True,
                    )
                    e_t = e_pool.tile([128, TQ], F16, name="e_t")
                    nc.scalar.activation(
                        e_t[:], st[:], mybir.ActivationFunctionType.Exp,
                        scale=SCALE,
                    )
                    if kt >= diag0:
                        # keep E[p,c] iff (t0+c) - (128*kt+p) >= 0
                        nc.gpsimd.affine_select(
                            out=e_t[:], in_=e_t[:],
                            compare_op=mybir.AluOpType.is_ge,
                            fill=0.0, base=t0 - 128 * kt,
                            pattern=[[1, TQ]], channel_multiplier=-1,
                        )
                    nc.tensor.matmul(
                        ot_ps[:], V[:, kt * 128:(kt + 1) * 128], e_t[:],
                        start=(i == 0), stop=(i == nkt - 1),
                    )
                    if i < 2:
                        nc.vector.tensor_copy((G0 if i == 0 else G1)[:], e_t[:])
                    elif i % 2 == 0:
                        nc.vector.tensor_add(G0[:], G0[:], e_t[:])
                    else:
                        nc.vector.tensor_add(G1[:], G1[:], e_t[:])

                ot_sb = ot_pool.tile([128, TQ], F32, name="ot_sb")
                nc.vector.tensor_copy(ot_sb[:], ot_ps[:])
                onat = onat_pool.tile([128, 4 * 128], F32, name="onat")
                dps = misc_psum.tile([128, 128], F32, tag="misc", name="dps")
                for c in range(TQ // 128):
                    nc.tensor.matmul(
                        dps[:, c:c + 1], G0[:, c * 128:(c + 1) * 128], ones[:],
                        start=True, stop=False,
                    )
                    nc.tensor.matmul(
                        dps[:, c:c + 1], G1[:, c * 128:(c + 1) * 128], ones[:],
                        start=False, stop=True,
                    )
                rc = small_pool.tile([128, 4], F32, name="rc")
                nc.vector.reciprocal(rc[:], dps[:, 0:4])
                for c in range(TQ // 128):
                    tp = misc_psum.tile([128, 128], F32, tag="misc", name="tp")
                    nc.tensor.transpose(
                        tp[:], ot_sb[:, c * 128:(c + 1) * 128], ident32[:]
                    )
                    nc.vector.tensor_scalar_mul(
                        onat[:, c * 128:(c + 1) * 128], tp[:], rc[:, c:c + 1]
                    )
                # one contiguous DMA per query tile (host un-permutes)
                nc.sync.dma_start(
                    out=out[j * 128:(j + 1) * 128, :], in_=onat[:],
                )

            # interleave: project each chunk, then run any attention tile
            # whose keys/queries are now fully projected
            done = set()
            for cc in range(kv_ccs):
                project(cc)
                for j, t0 in enumerate(t0s):
                    if j in done:
                        continue
                    # one chunk of slack after strictly-ready to avoid PE
                    # head-of-line stalls on exp/mask deps
                    if (t0 + TQ) // TQ <= cc and (t0 // TQ) < cc or cc == kv_ccs - 1:
                        done.add(j)
                        attend(j, t0)
            assert done == set(range(len(t0s)))

    nc.finalize()
    return nc


# ---------------- host-side run ----------------

_CACHE = {}


def _runner(nc, devices):
    """run_bass_via_pjrt with an explicit device list (subset launch)."""
    import jax
    from jax.sharding import Mesh, PartitionSpec
    from jax.experimental.shard_map import shard_map
    from concourse.bass2jax import _bass_exec_p, install_neuronx_cc_hook

    install_neuronx_cc_hook()
    n_cores = len(devices)
    part_name = nc.partition_id_tensor.name if nc.partition_id_tensor else None
    in_names, out_names, out_avals, zero_outs = [], [], [], []
    for alloc in nc.m.functions[0].allocations:
        if not isinstance(alloc, mybir.MemoryLocationSet):
            continue
        name = alloc.memorylocations[0].name
        if alloc.kind == "ExternalInput":
            if name != part_name:
                in_names.append(name)
        elif alloc.kind == "ExternalOutput":
            shape = tuple(alloc.tensor_shape)
            dtype = mybir.dt.np(alloc.dtype)
            out_names.append(name)
            out_avals.append(jax.core.ShapedArray(shape, dtype))
            zero_outs.append(np.zeros(shape, dtype))
    n_params = len(in_names)
    n_outs = len(out_avals)
    in_names = in_names + out_names
    if part_name is not None:
        in_names = in_names + [part_name]
    donate = tuple(range(n_params, n_params + n_outs))

    def _body(*args):
        from concourse.bass2jax import partition_id_tensor
        operands = list(args)
        if part_name is not None:
            operands.append(partition_id_tensor())
        outs = _bass_exec_p.bind(
            *operands,
            out_avals=tuple(out_avals),
            in_names=tuple(in_names),
            out_names=tuple(out_names),
            lowering_input_output_aliases=(),
            sim_require_finite=True,
            sim_require_nnan=True,
            nc=nc,
        )
        return tuple(outs)

    mesh = Mesh(np.asarray(devices), ("core",))
    sharded = jax.jit(
        shard_map(
            _body, mesh=mesh,
            in_specs=(PartitionSpec("core"),) * (n_params + n_outs),
            out_specs=(PartitionSpec("core"),) * n_outs,
            check_rep=False,
        ),
        donate_argnums=donate, keep_unused=True,
    )

    def run(in_maps):
        per_core = [[np.asarray(m[n]) for n in in_names[:n_params]] for m in in_maps]
        concat_in = [
            np.concatenate([per_core[c][i] for c in range(n_cores)], axis=0)
            for i in range(n_params)
        ]
        concat_zeros = [
            np.zeros((n_cores * z.shape[0], *z.shape[1:]), z.dtype) for z in zero_outs
        ]
        return sharded(*concat_in, *concat_zeros)

    def finish(out_arrs):
        return [
            {
                n: np.asarray(out_arrs[i]).reshape(n_cores, *out_avals[i].shape)[c]
                for i, n in enumerate(out_names)
            }
            for c in range(n_cores)
        ]

    return run, finish


def _pack_x(xb, kv_ccs):
    """x[b] [T,E] fp32 -> [kv_ccs*128, NE*512] fp16, tier rows."""
    xv = xb[:kv_ccs * TQ].reshape(kv_ccs, TQ, NE, 128)
    return np.ascontiguousarray(
        xv.transpose(0, 3, 2, 1).reshape(kv_ccs * 128, NE * TQ)
    ).astype(np.float16)


def _pack_w(w):
    """W [E,H] -> [128, NE*H] fp16 with w[p, e*H+h] = W[e*128+p, h]."""
    return np.ascontiguousarray(
        w.reshape(NE, 128, H).transpose(1, 0, 2).reshape(128, NE * H)
    ).astype(np.float16)


def _get_runners():
    if "runners" not in _CACHE:
        import jax
        devs = jax.devices()
        ncA = _build(T0S_A)
        ncB = _build(T0S_B)
        _CACHE["ncs"] = (ncA, ncB)
        runA = _runner(ncA, devs[0:4])
        runB = _runner(ncB, devs[4:8])
        # Warm each executable once, sequentially and blocking, before
        # any concurrent use (cold concurrent dispatch has raced before).
        zs = []
        for ccs in (8, 7):
            zs.append([
                {
                    "xp": np.zeros((ccs * 128, NE * TQ), np.float16),
                    "Wq": np.zeros((128, NE * H), np.float16),
                    "Wk": np.zeros((128, NE * H), np.float16),
                    "Wv": np.zeros((128, NE * H), np.float16),
                }
                for _ in range(B)
            ])
        for (run, fin), z in zip((runA, runB), zs):
            fin(run(z))
        _CACHE["runners"] = (runA, runB)
    return _CACHE["runners"]


def kernel(x, Wq, Wk, Wv):
    x = np.asarray(x, dtype=np.float32)
    (runA, finA), (runB, finB) = _get_runners()

    wq16, wk16, wv16 = (_pack_w(np.asarray(w)) for w in (Wq, Wk, Wv))
    mapsA = [
        {"xp": _pack_x(x[b], 8), "Wq": wq16, "Wk": wk16, "Wv": wv16}
        for b in range(B)
    ]
    mapsB = [
        {"xp": _pack_x(x[b], 7), "Wq": wq16, "Wk": wk16, "Wv": wv16}
        for b in range(B)
    ]
    # dispatch both meshes before blocking on either
    outA = runA(mapsA)
    outB = runB(mapsB)
    resA = finA(outA)
    resB = finB(outB)

    full = np.empty((B, T, H), np.float32)
    for b in range(B):
        for res, t0list in ((resA[b], T0S_A), (resB[b], T0S_B)):
            o = res["out"].reshape(NT, 128, 4, H).transpose(0, 2, 1, 3)
            for j, t0 in enumerate(t0list):
                full[b, t0:t0 + TQ] = o[j].reshape(TQ, H)
    return full


# revision 26
# speedup vs baseline: 1.1690x; 1.0406x over previous
"""Single-head causal attention on 8 Trainium2 NeuronCores.

B=4, T=4096, E=1024, H=128, fp32 in/out.

Sharding: batch-parallel x query-parallel. Two programs (one per query half):
  program A cores (devices 0-3): batch d, queries [0:1024) u [3072:4096)
  program B cores (devices 4-7): batch d-4, queries [1024:3072)
Per-core matmul unit counts (128x128x512 streams) balance at 296/296.

All DMAs are host-side pre-packed to be contiguous per partition:
  xp   [kv_ccs*NE*128, 512] fp16: block (cc,e) rows = x[b, cc*512+t, e*128+p]
  w*   [128, NE*H]  fp16: w[p, e*H+h] = W[e*128+p, h]
  out  [NT*128, 512] fp32: tile j rows p, cols c*128+h = O[t0_j + c*128 + p, h]

On-chip (per core); matmul operands fp16, accumulation fp32:
  0. PE warmup matmuls (identity) fill the initial DMA-wait window so the
     HAM clock gate reaches 8/8 before the real stream begins.
  1. QKV projections per 512-column chunk, 8 E-steps accumulated in PSUM
     ([128,1024] pair tiles hold K|V halves). V natural via PE transposes
     batched 4-per-bank then one DVE copy.
  2. Per 512-query tile, key tiles of 128 processed in PAIRS (diagonal
     pair first): two S^T matmuls into one [128,1024] PSUM pair, ONE exp
     (ScalarE) over 1024 cols -> fp16 SBUF, causal mask on diagonal tiles
     via gpsimd affine_select on halves, two PV matmuls accumulate into
     OT PSUM; G0 += half0 / G1 += half1 on DVE (fp16 2x mode).
  3. denom = colsum(G0)+colsum(G1) via ones-matmuls into PSUM, DVE
     reciprocal, PE-transpose OT chunks, row-scale by 1/denom, one
     contiguous DMA per query tile -> out [128, 512] fp32.
"""

import numpy as np

import concourse.bass as bass
import concourse.bacc as bacc
import concourse.mybir as mybir
import concourse.tile as tile
from concourse.masks import make_identity

B, T, E, H = 4, 4096, 1024, 128
TQ = 512          # query tile width
NE = E // 128     # 8 e-chunks
QROWS = 2048      # queries per core
NT = QROWS // TQ  # 4 query tiles per core
SCALE = float(H) ** -0.5
F32 = mybir.dt.float32
F16 = mybir.dt.float16

T0S_A = [0, 512, 2560, 3584]
T0S_B = [1024, 1536, 2048, 3072]
N_WARM = 36


def _build(t0s):
    nc = bacc.Bacc("TRN2", target_bir_lowering=False, debug=False, num_devices=4)
    kv_ccs = max((t0 + TQ) // TQ for t0 in t0s)       # 8 for A, 7 for B
    kv_cols = kv_ccs * TQ
    xp = nc.declare_dram_parameter("xp", [kv_ccs * 128, NE * TQ], F16, isOutput=False)
    Wq = nc.declare_dram_parameter("Wq", [128, NE * H], F16, isOutput=False)
    Wk = nc.declare_dram_parameter("Wk", [128, NE * H], F16, isOutput=False)
    Wv = nc.declare_dram_parameter("Wv", [128, NE * H], F16, isOutput=False)
    out = nc.declare_dram_parameter("out", [NT * 128, TQ], F32, isOutput=True)

    qcc = {t0 // TQ: j for j, t0 in enumerate(t0s)}  # T col-chunk -> q tile slot

    with tile.TileContext(nc) as tc:
        with (
            tc.tile_pool(name="const", bufs=1) as const_pool,
            tc.tile_pool(name="wts", bufs=1) as wt_pool,
            tc.tile_pool(name="big", bufs=1) as big_pool,
            tc.tile_pool(name="ev", bufs=6) as e_pool,
            tc.tile_pool(name="g", bufs=2) as g_pool,
            tc.tile_pool(name="ot", bufs=2) as ot_pool,
            tc.tile_pool(name="small", bufs=4) as small_pool,
            tc.tile_pool(name="onat", bufs=2) as onat_pool,
            tc.tile_pool(name="mmp", bufs=3, space="PSUM") as mm_psum,
            tc.tile_pool(name="pv", bufs=2, space="PSUM") as pv_psum,
            tc.tile_pool(name="tpv", bufs=1, space="PSUM") as tpv_psum,
            tc.tile_pool(name="misc", bufs=2, space="PSUM") as misc_psum,
        ):
            ident16 = const_pool.tile([128, 128], F16, tag="id16")
            make_identity(nc, ident16[:])
            ident32 = const_pool.tile([128, 128], F32, tag="id32")
            make_identity(nc, ident32[:])
            ones = const_pool.tile([128, 1], F16, tag="ones")
            nc.gpsimd.memset(ones[:], 1.0)

            # PE warmup: keep the array busy through the HAM window while
            # the first input DMAs land.
            warm_ps = tpv_psum.tile([128, TQ], F32, tag="tpv", name="warm")
            for i in range(N_WARM):
                nc.tensor.matmul(
                    warm_ps[:, 0:128], ident16[:], ident16[:],
                    start=(i == 0), stop=(i == N_WARM - 1),
                )

            # weights (host pre-packed): single contiguous DMA each
            wq_sb = wt_pool.tile([128, NE * H], F16, tag="wq")
            wk_sb = wt_pool.tile([128, NE * H], F16, tag="wk")
            wv_sb = wt_pool.tile([128, NE * H], F16, tag="wv")
            nc.sync.dma_start(out=wk_sb[:], in_=Wk[:])

            # x tiers (host pre-packed, contiguous rows): one DMA per tier,
            # all on the sync DGE in consumption order -- per-queue FIFO
            # means tier k's descriptors complete before tier k+1's, so the
            # first tiers land at full HBM bandwidth. Tier 0 is split so the
            # first projection matmuls can start even earlier.
            xts = []  # per cc: list of (e_lo, n_e, tile)
            for cc in range(kv_ccs):
                parts = ((0, 2), (2, 2), (4, 4)) if cc == 0 else ((0, NE),)
                tiles = []
                for e_lo, n_e in parts:
                    xt = big_pool.tile([128, n_e * TQ], F16,
                                       tag=f"xt{cc}_{e_lo}")
                    r0 = cc * 128
                    nc.sync.dma_start(
                        out=xt[:],
                        in_=xp[r0:r0 + 128,
                               e_lo * TQ:(e_lo + n_e) * TQ],
                    )
                    tiles.append((e_lo, n_e, xt))
                xts.append(tiles)
                if cc == 0:
                    # wv/wq land after tier0 (V/Q projections run later)
                    nc.sync.dma_start(out=wv_sb[:], in_=Wv[:])
                    nc.scalar.dma_start(out=wq_sb[:], in_=Wq[:])

            def xslice(cc, e):
                for e_lo, n_e, xt in xts[cc]:
                    if e_lo <= e < e_lo + n_e:
                        return xt[:, (e - e_lo) * TQ:(e - e_lo + 1) * TQ]
                raise AssertionError

            KT = big_pool.tile([128, kv_cols], F16, tag="kt")   # K^T
            VT = big_pool.tile([128, kv_cols], F16, tag="vt")   # V^T
            V = big_pool.tile([128, kv_cols], F16, tag="v")     # V natural
            QT = big_pool.tile([128, QROWS], F16, tag="qt")     # Q^T

            def project(cc):
                c0 = cc * TQ
                for w_sb, dstbuf, d0, cp in (
                    (wk_sb, KT, c0, "scalar" if cc % 2 == 0 else "vector"),
                    (wv_sb, VT, c0, "vector" if cc % 2 == 0 else "scalar"),
                ) + (((wq_sb, QT, qcc[cc] * TQ, "vector"),) if cc in qcc else ()):
                    ps = mm_psum.tile([128, TQ], F32, tag="mm", name="ps")
                    for e in range(NE):
                        nc.tensor.matmul(
                            ps[:], w_sb[:, e * H:(e + 1) * H], xslice(cc, e),
                            start=(e == 0), stop=(e == NE - 1),
                        )
                    if cp == "scalar":
                        nc.scalar.copy(dstbuf[:, d0:d0 + TQ], ps[:])
                    else:
                        nc.vector.tensor_copy(dstbuf[:, d0:d0 + TQ], ps[:])
                # V natural: 4 PE transposes into one bank, one DVE copy out
                tp16 = tpv_psum.tile([128, TQ], F16, tag="tpv", name="tp16")
                for i in range(4):
                    kt = cc * 4 + i
                    nc.tensor.transpose(
                        tp16[:, i * 128:(i + 1) * 128],
                        VT[:, kt * 128:(kt + 1) * 128], ident16[:],
                    )
                nc.vector.tensor_copy(V[:, c0:c0 + TQ], tp16[:])

            def attend(j, t0):
                nkt = (t0 + TQ) // 128
                diag0 = t0 // 128
                ot_ps = pv_psum.tile([128, TQ], F32, name="ot_ps")
                G0 = g_pool.tile([128, TQ], F16, tag="g0", name="G0")
                G1 = g_pool.tile([128, TQ], F16, tag="g1", name="G1")
                kts = list(range(nkt - 1, -1, -1))  # diagonal first
                for i, kt in enumerate(kts):
                    st = mm_psum.tile([128, TQ], F32, tag="mm", name="st")
                    nc.tensor.matmul(
                        st[:], KT[:, kt * 128:(kt + 1) * 128],
                        QT[:, j * TQ:(j + 1) * TQ],
                        start=True, stop=True,
                    )
                    e_t = e_pool.tile([128, TQ], F16, name="e_t")
                    nc.scalar.activation(
                        e_t[:], st[:], mybir.ActivationFunctionType.Exp,
                        scale=SCALE,
                    )
                    if kt >= diag0:
                        # keep E[p,c] iff (t0+c) - (128*kt+p) >= 0
                        nc.gpsimd.affine_select(
                            out=e_t[:], in_=e_t[:],
                            compare_op=mybir.AluOpType.is_ge,
                            fill=0.0, base=t0 - 128 * kt,
                            pattern=[[1, TQ]], channel_multiplier=-1,
                        )
                    nc.tensor.matmul(
                        ot_ps[:], V[:, kt * 128:(kt + 1) * 128], e_t[:],
                        start=(i == 0), stop=(i == nkt - 1),
                    )
                    if i < 2:
                        nc.vector.tensor_copy((G0 if i == 0 else G1)[:], e_t[:])
                    elif i % 2 == 0:
                        nc.vector.tensor_add(G0[:], G0[:], e_t[:])
                    else:
                        nc.vector.tensor_add(G1[:], G1[:], e_t[:])

                ot_sb = ot_pool.tile([128, TQ], F32, name="ot_sb")
                nc.vector.tensor_copy(ot_sb[:], ot_ps[:])
                onat = onat_pool.tile([128, 4 * 128], F32, name="onat")
                dps = misc_psum.tile([128, 128], F32, tag="misc", name="dps")
                for c in range(TQ // 128):
                    nc.tensor.matmul(
                        dps[:, c:c + 1], G0[:, c * 128:(c + 1) * 128], ones[:],
                        start=True, stop=False,
                    )
                    nc.tensor.matmul(
                        dps[:, c:c + 1], G1[:, c * 128:(c + 1) * 128], ones[:],
                        start=False, stop=True,
                    )
                rc = small_pool.tile([128, 4], F32, name="rc")
                nc.vector.reciprocal(rc[:], dps[:, 0:4])
                for c in range(TQ // 128):
                    tp = misc_psum.tile([128, 128], F32, tag="misc", name="tp")
                    nc.tensor.transpose(
                        tp[:], ot_sb[:, c * 128:(c + 1) * 128], ident32[:]
                    )
                    nc.vector.tensor_scalar_mul(
                        onat[:, c * 128:(c + 1) * 128], tp[:], rc[:, c:c + 1]
                    )
                # one contiguous DMA per query tile (host un-permutes)
                nc.sync.dma_start(
                    out=out[j * 128:(j + 1) * 128, :], in_=onat[:],
                )

            # interleave: project each chunk, then run any attention tile
            # whose keys/queries are now fully projected
            done = set()
            for cc in range(kv_ccs):
                project(cc)
                for j, t0 in enumerate(t0s):
                    if j in done:
                        continue
                    # one chunk of slack after strictly-ready to avoid PE
                    # head-of-line stalls on exp/mask deps
                    if (t0 + TQ) // TQ <= cc and (t0 // TQ) < cc or cc == kv_ccs - 1:
                        done.add(j)
                        attend(j, t0)
            assert done == set(range(len(t0s)))

    nc.finalize()
    return nc


# ---------------- host-side run ----------------

_CACHE = {}


def _runner(nc, devices):
    """run_bass_via_pjrt with an explicit device list (subset launch)."""
    import jax
    from jax.sharding import Mesh, PartitionSpec
    from jax.experimental.shard_map import shard_map
    from concourse.bass2jax import _bass_exec_p, install_neuronx_cc_hook

    install_neuronx_cc_hook()
    n_cores = len(devices)
    part_name = nc.partition_id_tensor.name if nc.partition_id_tensor else None
    in_names, out_names, out_avals, zero_outs = [], [], [], []
    for alloc in nc.m.functions[0].allocations:
        if not isinstance(alloc, mybir.MemoryLocationSet):
            continue
        name = alloc.memorylocations[0].name
        if alloc.kind == "ExternalInput":
            if name != part_name:
                in_names.append(name)
        elif alloc.kind == "ExternalOutput":
            shape = tuple(alloc.tensor_shape)
            dtype = mybir.dt.np(alloc.dtype)
            out_names.append(name)
            out_avals.append(jax.core.ShapedArray(shape, dtype))
            zero_outs.append(np.zeros(shape, dtype))
    n_params = len(in_names)
    n_outs = len(out_avals)
    in_names = in_names + out_names
    if part_name is not None:
        in_names = in_names + [part_name]
    donate = tuple(range(n_params, n_params + n_outs))

    def _body(*args):
        from concourse.bass2jax import partition_id_tensor
        operands = list(args)
        if part_name is not None:
            operands.append(partition_id_tensor())
        outs = _bass_exec_p.bind(
            *operands,
            out_avals=tuple(out_avals),
            in_names=tuple(in_names),
            out_names=tuple(out_names),
            lowering_input_output_aliases=(),
            sim_require_finite=True,
            sim_require_nnan=True,
            nc=nc,
        )
        return tuple(outs)

    mesh = Mesh(np.asarray(devices), ("core",))
    sharded = jax.jit(
        shard_map(
            _body, mesh=mesh,
            in_specs=(PartitionSpec("core"),) * (n_params + n_outs),
            out_specs=(PartitionSpec("core"),) * n_outs,
            check_rep=False,
        ),
        donate_argnums=donate, keep_unused=True,
    )

    def run(in_maps):
        per_core = [[np.asarray(m[n]) for n in in_names[:n_params]] for m in in_maps]
        concat_in = [
            np.concatenate([per_core[c][i] for c in range(n_cores)], axis=0)
            for i in range(n_params)
        ]
        concat_zeros = [
            np.zeros((n_cores * z.shape[0], *z.shape[1:]), z.dtype) for z in zero_outs
        ]
        return sharded(*concat_in, *concat_zeros)

    def finish(out_arrs):
        return [
            {
                n: np.asarray(out_arrs[i]).reshape(n_cores, *out_avals[i].shape)[c]
                for i, n in enumerate(out_names)
            }
            for c in range(n_cores)
        ]

    return run, finish


def _pack_x(xb, kv_ccs):
    """x[b] [T,E] fp32 -> [kv_ccs*128, NE*512] fp16, tier rows."""
    xv = xb[:kv_ccs * TQ].reshape(kv_ccs, TQ, NE, 128)
    return np.ascontiguousarray(
        xv.transpose(0, 3, 2, 1).reshape(kv_ccs * 128, NE * TQ)
    ).astype(np.float16)


def _pack_w(w):
    """W [E,H] -> [128, NE*H] fp16 with w[p, e*H+h] = W[e*128+p, h]."""
    return np.ascontiguousarray(
        w.reshape(NE, 128, H).transpose(1, 0, 2).reshape(128, NE * H)
    ).astype(np.float16)


def _get_runners():
    if "runners" not in _CACHE:
        import jax
        devs = jax.devices()
        ncA = _build(T0S_A)
        ncB = _build(T0S_B)
        _CACHE["ncs"] = (ncA, ncB)
        runA = _runner(ncA, devs[0:4])
        runB = _runner(ncB, devs[4:8])
        # Warm each executable once, sequentially and blocking, before
        # any concurrent use (cold concurrent dispatch has raced before).
        zs = []
        for ccs in (8, 7):
            zs.append([
                {
                    "xp": np.zeros((ccs * 128, NE * TQ), np.float16),
                    "Wq": np.zeros((128, NE * H), np.float16),
                    "Wk": np.zeros((128, NE * H), np.float16),
                    "Wv": np.zeros((128, NE * H), np.float16),
                }
                for _ in range(B)
            ])
        for (run, fin), z in zip((runA, runB), zs):
            fin(run(z))
        _CACHE["runners"] = (runA, runB)
    return _CACHE["runners"]


def kernel(x, Wq, Wk, Wv):
    x = np.asarray(x, dtype=np.float32)
    (runA, finA), (runB, finB) = _get_runners()

    wq16, wk16, wv16 = (_pack_w(np.asarray(w)) for w in (Wq, Wk, Wv))
    mapsA = [
        {"xp": _pack_x(x[b], 8), "Wq": wq16, "Wk": wk16, "Wv": wv16}
        for b in range(B)
    ]
    mapsB = [
        {"xp": _pack_x(x[b], 7), "Wq": wq16, "Wk": wk16, "Wv": wv16}
        for b in range(B)
    ]
    # dispatch both meshes before blocking on either
    outA = runA(mapsA)
    outB = runB(mapsB)
    resA = finA(outA)
    resB = finB(outB)

    full = np.empty((B, T, H), np.float32)
    for b in range(B):
        for res, t0list in ((resA[b], T0S_A), (resB[b], T0S_B)):
            o = res["out"].reshape(NT, 128, 4, H).transpose(0, 2, 1, 3)
            for j, t0 in enumerate(t0list):
                full[b, t0:t0 + TQ] = o[j].reshape(TQ, H)
    return full
